# revision 30
# baseline (speedup 1.0000x reference)
"""Grouped-experts SwiGLU MLP (DeepseekV3 style) for Trainium2, 8 NeuronCores.

Sharding: expert-parallel. Core e owns expert e's weights and its static
4096-token split. No collectives needed — token routing is the host-side
slice, outputs concatenate back in token order.

Per-core kernel (all matmuls in bf16 with fp32 PSUM accumulation):
  gT[h, t] = wg[d, h].T @ xT[d, t]      (accumulate over 16 d-chunks of 128)
  uT[h, t] = wu[d, h].T @ xT[d, t]
  hT[h, t] = silu(gT) * uT              (ACT sigmoid + DVE muls, stored bf16)
  out[t, d] = hT[h, t].T @ wd[h, d]     (accumulate over 11 h-chunks of 128)

The PE is the bottleneck (bf16 matmul roofline ~900us/core; fp8 DoubleRow
fails the 2e-2 accuracy budget — measured 3.8-6.6% rel err on this problem).
So everything else is shaped to keep the PE issue stream stall-free:

- All HBM tensors are bf16, cast host-side (identical numerics to the old
  fp32->bf16 DMA cast path, RNE both ways). Halves startup DMA bytes: the
  PE's first accumulation chain needs wg + x(group 0), 7.7 MB instead of
  15.7 MB, and the full weight set streams in 48us instead of 97us.
- Inputs land in per-contraction-chunk tiles (16 separate wg/wu/x tiles)
  DMA'd in consumption order, wg/x interleaved across two issue engines, so
  the first matmul starts right after the ~7us engine preamble and group 0
  is paced by DMA arrival rather than blocked on the whole tensor.
- Group 0 warm-start: 6 accumulation chains run chunk-OUTER across 6 PSUM
  banks (borrowing the idle pu/po-tagged banks), so every arriving chunk
  feeds 6 matmuls instead of the PE idling behind one 16-matmul chain.
  NOTE: >=8 interleaved chains (or a bare nc.tensor.ldweights) makes the
  compiler drop Fast Weight Load globally: every LDWEIGHTS doubles to
  ~107ns, matmul cadence goes 215->265ns, +170us whole-kernel. Measured.
- Down-projection runs in dim-halves with po double-buffered (2+2 PSUM
  banks) and bf16 out tiles double-buffered, so the PSUM->bf16 casts
  (690ns each on DVE) and output stores pipeline against the matmuls
  instead of stalling every tb boundary (was ~430ns/tb mid-kernel and
  ~3us/tb in the last group).
- Output is bf16 (adds ~0.02% rel err), upcast to fp32 host-side.

Measured on 8xTRN2 (NTFF profile): 941 us/core, PE matmul-issue floor is
908 us (4224 matmuls x 215 ns), MFU ~0.87. Residual overhead: ~7us engine
preamble, ~5us group-0 DMA pacing, ~4us instruction-fetch bubbles (432ns
per 100 PE instructions), ~5us end drain. fp8 DoubleRow would cut the
floor ~1.44x but fails the 2e-2 budget: e4m3 all-fp8 measures 6.6% rel
err, best hybrid 3.8%; e3m4 (not DoubleRow-capable) 2.0-3.4%.
"""

import numpy as np

NUM_EXPERTS = 8
DIM = 2048
HIDDEN = 1408
T_E = 4096  # tokens per expert (static equal splits)

P = 128
TN = 512              # token group width (matmul moving dim)
NG = T_E // TN        # 8 token groups
DC = DIM // P         # 16 contraction chunks for the up/gate matmuls
HC = HIDDEN // P      # 11 contraction chunks for the down matmul
NDO = DIM // TN       # 4 output-dim blocks of 512

_nc_cache = []


def _build_program():
    import concourse.mybir as mybir
    import concourse.tile as tile
    from concourse import bacc

    fp32 = mybir.dt.float32
    bf16 = mybir.dt.bfloat16
    AF = mybir.ActivationFunctionType

    nc = bacc.Bacc("TRN2", target_bir_lowering=False, debug=False)

    xt = nc.dram_tensor("xt", [NG, DC, P, TN], bf16, kind="ExternalInput")
    wg = nc.dram_tensor("wg", [DC, P, HIDDEN], bf16, kind="ExternalInput")
    wu = nc.dram_tensor("wu", [DC, P, HIDDEN], bf16, kind="ExternalInput")
    wd = nc.dram_tensor("wd", [HC, P, DIM], bf16, kind="ExternalInput")
    out = nc.dram_tensor("out", [T_E, DIM], bf16, kind="ExternalOutput")

    with tile.TileContext(nc) as tc:
        with (
            tc.tile_pool(name="wpool", bufs=1) as wpool,
            tc.tile_pool(name="xpool", bufs=2) as xpool,
            tc.tile_pool(name="hpool", bufs=1) as hpool,
            tc.tile_pool(name="spool", bufs=1) as spool,
            tc.tile_pool(name="opool", bufs=2) as opool,
            tc.tile_pool(name="psum", bufs=2, space="PSUM") as psum_pool,
        ):
            # wg chunk 0 is split: the warm-start's first LDWEIGHTS needs
            # only hh 0..5 (cols 0:768), so a 192KB slice gates the first
            # matmul instead of the full 352KB chunk.
            W0A = 6 * P
            wg0a = wpool.tile([P, W0A], bf16, tag="wg0a")
            wg0b = wpool.tile([P, HIDDEN - W0A], bf16, tag="wg0b")
            wg_sb = [None] + [
                wpool.tile([P, HIDDEN], bf16, tag=f"wg{c}", name=f"wg{c}")
                for c in range(1, DC)
            ]

            def wg_slice(c, hh):
                if c == 0:
                    if hh < 6:
                        return wg0a[:, hh * P:(hh + 1) * P]
                    return wg0b[:, (hh - 6) * P:(hh - 5) * P]
                return wg_sb[c][:, hh * P:(hh + 1) * P]
            wu_sb = [
                wpool.tile([P, HIDDEN], bf16, tag=f"wu{c}", name=f"wu{c}")
                for c in range(DC)
            ]
            wd_sb = wpool.tile([P, HC, DIM], bf16, tag="wd")
            x0_sb = [
                xpool.tile([P, TN], bf16, tag=f"x{c}", name=f"x0_{c}")
                for c in range(DC)
            ]
            # Consumption-ordered loads, interleaved across two issue engines
            # so descriptor emission (~0.65us/DMA/engine) doesn't pace the
            # queues. wg/x0 pace the warm-start PE chains, so they go first;
            # wu (needed ~32us in, by the u-pass) is split across both
            # engines behind them, then wd (needed ~75us in).
            nc.gpsimd.dma_start(out=wg0a, in_=wg[0][:, 0:W0A])
            nc.sync.dma_start(out=x0_sb[0], in_=xt[0, 0])
            for c in range(1, DC):
                nc.gpsimd.dma_start(out=wg_sb[c], in_=wg[c])
                nc.sync.dma_start(out=x0_sb[c], in_=xt[0, c])
            nc.gpsimd.dma_start(out=wg0b, in_=wg[0][:, W0A:])
            for c in range(DC // 2):
                nc.gpsimd.dma_start(out=wu_sb[c], in_=wu[c])
            for c in range(DC // 2, DC):
                nc.sync.dma_start(out=wu_sb[c], in_=wu[c])
            # wd is NOT enqueued here: its dma_starts sit in the scalar
            # engine's stream behind the warm-start sigmoids (below), so the
            # transfers only begin once the g-pass is done and stop competing
            # for HBM bandwidth with the wg/x/wu chunks that gate the PE.

            for g in range(NG):
                if g == 0:
                    x_sb = x0_sb
                else:
                    x_sb = [
                        xpool.tile([P, TN], bf16, tag=f"x{c}", name=f"x{g}_{c}")
                        for c in range(DC)
                    ]
                    for c in range(DC):
                        nc.gpsimd.dma_start(out=x_sb[c], in_=xt[g, c])

                ht_sb = hpool.tile([P, HC, TN], bf16, tag="ht")

                if g == 0:
                    # Warm-start: group 0's weights/x stream in while the PE
                    # runs, so a chunk-inner chain (16 matmuls per hh on one
                    # PSUM bank) would leave the PE idle whenever it outruns
                    # DMA arrival. Instead run 6 accumulation chains
                    # (hh 0..5) chunk-OUTER across 6 PSUM banks (borrowing
                    # the pu/po-tagged banks, idle until later): every
                    # arriving chunk feeds 6 matmuls, first for w_gate, then
                    # the same shape again for w_up, with silu(g) staged to
                    # SBUF between the passes. hh 6..10 then take the
                    # standard path below. 6 chains, not 8: 8 interleaved
                    # accumulation groups make the compiler drop FWL
                    # globally (see module docstring).
                    NW = 6
                    sil8 = spool.tile([P, NW, TN], bf16, tag="sil8")
                    for wsb, drain in ((wg_sb, "g"), (wu_sb, "u")):
                        ch_a = psum_pool.tile([P, TN], fp32, tag="pg", name=f"w0{drain}a")
                        ch_b = psum_pool.tile([P, TN], fp32, tag="pg", name=f"w0{drain}b")
                        ch_c = psum_pool.tile([P, TN], fp32, tag="pu", name=f"w0{drain}c")
                        ch_d = psum_pool.tile([P, TN], fp32, tag="pu", name=f"w0{drain}d")
                        ch_ef = psum_pool.tile([P, 2, TN], fp32, tag="po", name=f"w0{drain}ef")
                        chains = [ch_a, ch_b, ch_c, ch_d, ch_ef[:, 0, :], ch_ef[:, 1, :]]
                        for c in range(DC):
                            for hh in range(NW):
                                nc.tensor.matmul(
                                    chains[hh],
                                    wg_slice(c, hh) if drain == "g"
                                    else wsb[c][:, hh * P:(hh + 1) * P],
                                    x_sb[c],
                                    start=(c == 0),
                                    stop=(c == DC - 1),
                                )
                        if drain == "g":
                            for hh in range(NW):
                                sg = spool.tile([P, TN], fp32, tag="sig", name=f"w0sg{hh}")
                                nc.scalar.activation(sg, chains[hh], AF.Sigmoid)
                                nc.vector.tensor_mul(sil8[:, hh, :], chains[hh], sg)
                            for h in range(HC):
                                nc.scalar.dma_start(out=wd_sb[:, h, :], in_=wd[h])
                        else:
                            for hh in range(NW):
                                nc.vector.tensor_mul(
                                    ht_sb[:, hh, :], sil8[:, hh, :], chains[hh]
                                )
                    hh_range = range(NW, HC)
                else:
                    hh_range = range(HC)

                for hh in hh_range:
                    pg = psum_pool.tile([P, TN], fp32, tag="pg")
                    pu = psum_pool.tile([P, TN], fp32, tag="pu")
                    for c in range(DC):
                        nc.tensor.matmul(
                            pg,
                            wg_slice(c, hh),
                            x_sb[c],
                            start=(c == 0),
                            stop=(c == DC - 1),
                        )
                    for c in range(DC):
                        nc.tensor.matmul(
                            pu,
                            wu_sb[c][:, hh * P:(hh + 1) * P],
                            x_sb[c],
                            start=(c == 0),
                            stop=(c == DC - 1),
                        )
                    # silu(g)*u = (g * sigmoid(g)) * u. Each DVE op reads at
                    # most one PSUM operand (HW limit NCC_IBVF027); Silu LUT
                    # isn't in CoreSim so sigmoid+mul keeps this sim-testable.
                    sig = spool.tile([P, TN], fp32, tag="sig")
                    sil = spool.tile([P, TN], fp32, tag="sil")
                    nc.scalar.activation(sig, pg, AF.Sigmoid)
                    nc.vector.tensor_mul(sil, pg, sig)
                    nc.vector.tensor_mul(ht_sb[:, hh, :], sil, pu)

                # Down-projection in dim-halves: po spans 2 PSUM banks and is
                # double-buffered (4 banks total; pg/pu take the other 4), so
                # the PSUM->bf16 casts and the out store of one half overlap
                # the matmuls of the next instead of stalling the PE at every
                # tb boundary (LDWEIGHTS is emitted 1:1 per matmul by the
                # framework, so the extra half split costs no weight loads).
                HD = DIM // 2
                for tb in range(TN // P):
                    t0 = g * TN + tb * P
                    for half in range(2):
                        ot = opool.tile([P, HD], bf16, tag="ot", name=f"ot{g}_{tb}_{half}")
                        po = psum_pool.tile([P, 2, TN], fp32, tag="po")
                        last = g == NG - 1 and tb == TN // P - 1 and half == 1
                        if not last:
                            for hh in range(HC):
                                for do in range(2):
                                    nc.tensor.matmul(
                                        po[:, do, :],
                                        ht_sb[:, hh, tb * P:(tb + 1) * P],
                                        wd_sb[:, hh, (half * 2 + do) * TN:(half * 2 + do + 1) * TN],
                                        start=(hh == 0),
                                        stop=(hh == HC - 1),
                                    )
                        if last:
                            # Final half: run the two do-chains sequentially
                            # (instead of hh-interleaved) and drain each as
                            # it stops, so cast+store of do=0 overlap the
                            # do=1 matmul chain and only one 690ns cast plus
                            # a 128KB store trail the last matmul.
                            for do in range(2):
                                for hh in range(HC):
                                    nc.tensor.matmul(
                                        po[:, do, :],
                                        ht_sb[:, hh, tb * P:(tb + 1) * P],
                                        wd_sb[:, hh, (half * 2 + do) * TN:(half * 2 + do + 1) * TN],
                                        start=(hh == 0),
                                        stop=(hh == HC - 1),
                                        skip_group_check=True,
                                    )
                                nc.vector.tensor_copy(
                                    ot[:, do * TN:(do + 1) * TN], po[:, do, :]
                                )
                                nc.sync.dma_start(
                                    out=out[t0:t0 + P,
                                            half * HD + do * TN:half * HD + (do + 1) * TN],
                                    in_=ot[:, do * TN:(do + 1) * TN],
                                )
                        else:
                            for do in range(2):
                                nc.vector.tensor_copy(ot[:, do * TN:(do + 1) * TN], po[:, do, :])
                            nc.sync.dma_start(
                                out=out[t0:t0 + P, half * HD:(half + 1) * HD], in_=ot
                            )

    nc.compile()
    return nc


def _get_program():
    if not _nc_cache:
        _nc_cache.append(_build_program())
    return _nc_cache[0]


def _in_map_for_core(xe, w_gate, w_up, w_down, e):
    import ml_dtypes

    bf = ml_dtypes.bfloat16
    xtc = np.ascontiguousarray(xe[e].T).astype(bf)           # [DIM, T_E]
    xtc = np.ascontiguousarray(
        xtc.reshape(DC, P, NG, TN).transpose(2, 0, 1, 3)     # [NG, DC, P, TN]
    )
    return {
        "xt": xtc,
        "wg": np.ascontiguousarray(w_gate[e].astype(bf).reshape(DC, P, HIDDEN)),
        "wu": np.ascontiguousarray(w_up[e].astype(bf).reshape(DC, P, HIDDEN)),
        "wd": np.ascontiguousarray(w_down[e].astype(bf).reshape(HC, P, DIM)),
    }


def kernel(x, num_tokens_per_expert, w_gate, w_up, w_down, **_ignored):
    from concourse.bass_utils import run_bass_kernel_spmd

    x = np.asarray(x, dtype=np.float32)
    w_gate = np.asarray(w_gate, dtype=np.float32)
    w_up = np.asarray(w_up, dtype=np.float32)
    w_down = np.asarray(w_down, dtype=np.float32)

    nc = _get_program()

    xe = x.reshape(NUM_EXPERTS, T_E, DIM)
    in_maps = [
        _in_map_for_core(xe, w_gate, w_up, w_down, e) for e in range(NUM_EXPERTS)
    ]

    res = run_bass_kernel_spmd(nc, in_maps, core_ids=list(range(NUM_EXPERTS)))
    outs = [np.asarray(r["out"]).astype(np.float32) for r in res.results]
    return np.concatenate(outs, axis=0)


# revision 32
# speedup vs baseline: 1.0224x; 1.0224x over previous
"""Grouped-experts SwiGLU MLP (DeepseekV3 style) for Trainium2, 8 NeuronCores.

Sharding: expert-parallel. Core e owns expert e's weights and its static
4096-token split. No collectives needed — token routing is the host-side
slice, outputs concatenate back in token order.

Per-core kernel (all matmuls in bf16 with fp32 PSUM accumulation):
  gT[h, t] = wg[d, h].T @ xT[d, t]      (accumulate over 16 d-chunks of 128)
  uT[h, t] = wu[d, h].T @ xT[d, t]
  hT[h, t] = silu(gT) * uT              (ACT sigmoid + DVE muls, stored bf16)
  out[t, d] = hT[h, t].T @ wd[h, d]     (accumulate over 11 h-chunks of 128)

The PE is the bottleneck (bf16 matmul roofline ~900us/core; fp8 DoubleRow
fails the 2e-2 accuracy budget — measured 3.8-6.6% rel err on this problem).
So everything else is shaped to keep the PE issue stream stall-free:

- All HBM tensors are bf16, cast host-side (identical numerics to the old
  fp32->bf16 DMA cast path, RNE both ways). Halves startup DMA bytes: the
  PE's first accumulation chain needs wg + x(group 0), 7.7 MB instead of
  15.7 MB, and the full weight set streams in 48us instead of 97us.
- Inputs land in per-contraction-chunk tiles (16 separate wg/wu/x tiles)
  DMA'd in consumption order, wg/x interleaved across two issue engines, so
  the first matmul starts right after the ~7us engine preamble and group 0
  is paced by DMA arrival rather than blocked on the whole tensor.
- Group 0 warm-start: 6 accumulation chains run chunk-OUTER across 6 PSUM
  banks (borrowing the idle pu/po-tagged banks), so every arriving chunk
  feeds 6 matmuls instead of the PE idling behind one 16-matmul chain.
  NOTE: >=8 interleaved chains (or a bare nc.tensor.ldweights) makes the
  compiler drop Fast Weight Load globally: every LDWEIGHTS doubles to
  ~107ns, matmul cadence goes 215->265ns, +170us whole-kernel. Measured.
- Down-projection runs in dim-halves with po double-buffered (2+2 PSUM
  banks) and bf16 out tiles double-buffered, so the PSUM->bf16 casts
  (690ns each on DVE) and output stores pipeline against the matmuls
  instead of stalling every tb boundary (was ~430ns/tb mid-kernel and
  ~3us/tb in the last group).
- Output is bf16 (adds ~0.02% rel err), upcast to fp32 host-side.

Measured on 8xTRN2 (NTFF profile): 941 us/core, PE matmul-issue floor is
908 us (4224 matmuls x 215 ns), MFU ~0.87. Residual overhead: ~7us engine
preamble, ~5us group-0 DMA pacing, ~4us instruction-fetch bubbles (432ns
per 100 PE instructions), ~5us end drain. fp8 DoubleRow would cut the
floor ~1.44x but fails the 2e-2 budget: e4m3 all-fp8 measures 6.6% rel
err, best hybrid 3.8%; e3m4 (not DoubleRow-capable) 2.0-3.4%.
"""

import numpy as np

NUM_EXPERTS = 8
DIM = 2048
HIDDEN = 1408
T_E = 4096  # tokens per expert (static equal splits)

P = 128
TN = 512              # token group width (matmul moving dim)
NG = T_E // TN        # 8 token groups
DC = DIM // P         # 16 contraction chunks for the up/gate matmuls
HC = HIDDEN // P      # 11 contraction chunks for the down matmul
NDO = DIM // TN       # 4 output-dim blocks of 512

_nc_cache = []


def _build_program():
    import concourse.mybir as mybir
    import concourse.tile as tile
    from concourse import bacc

    fp32 = mybir.dt.float32
    bf16 = mybir.dt.bfloat16
    AF = mybir.ActivationFunctionType

    nc = bacc.Bacc("TRN2", target_bir_lowering=False, debug=False)

    xt = nc.dram_tensor("xt", [NG, DC, P, TN], bf16, kind="ExternalInput")
    wg = nc.dram_tensor("wg", [DC, P, HIDDEN], bf16, kind="ExternalInput")
    wu = nc.dram_tensor("wu", [DC, P, HIDDEN], bf16, kind="ExternalInput")
    wd = nc.dram_tensor("wd", [HC, P, DIM], bf16, kind="ExternalInput")
    out = nc.dram_tensor("out", [T_E, DIM], bf16, kind="ExternalOutput")

    with tile.TileContext(nc) as tc:
        with (
            tc.tile_pool(name="wpool", bufs=1) as wpool,
            tc.tile_pool(name="xpool", bufs=2) as xpool,
            tc.tile_pool(name="hpool", bufs=1) as hpool,
            tc.tile_pool(name="spool", bufs=1) as spool,
            tc.tile_pool(name="opool", bufs=2) as opool,
            tc.tile_pool(name="psum", bufs=2, space="PSUM") as psum_pool,
        ):
            # wg chunk 0 is split: the warm-start's first LDWEIGHTS needs
            # only hh 0..5 (cols 0:768), so a 192KB slice gates the first
            # matmul instead of the full 352KB chunk.
            W0A = 6 * P
            wg0a = wpool.tile([P, W0A], bf16, tag="wg0a")
            wg0b = wpool.tile([P, HIDDEN - W0A], bf16, tag="wg0b")
            wg_sb = [None] + [
                wpool.tile([P, HIDDEN], bf16, tag=f"wg{c}", name=f"wg{c}")
                for c in range(1, DC)
            ]

            def wg_slice(c, hh):
                if c == 0:
                    if hh < 6:
                        return wg0a[:, hh * P:(hh + 1) * P]
                    return wg0b[:, (hh - 6) * P:(hh - 5) * P]
                return wg_sb[c][:, hh * P:(hh + 1) * P]
            wu_sb = [
                wpool.tile([P, HIDDEN], bf16, tag=f"wu{c}", name=f"wu{c}")
                for c in range(DC)
            ]
            wd_sb = wpool.tile([P, HC, DIM], bf16, tag="wd")
            x0_sb = [
                xpool.tile([P, TN], bf16, tag=f"x{c}", name=f"x0_{c}")
                for c in range(DC)
            ]
            # Consumption-ordered loads, interleaved across two issue engines
            # so descriptor emission (~0.65us/DMA/engine) doesn't pace the
            # queues. wg/x0 pace the warm-start PE chains, so they go first;
            # wu (needed ~32us in, by the u-pass) is split across both
            # engines behind them, then wd (needed ~75us in).
            nc.gpsimd.dma_start(out=wg0a, in_=wg[0][:, 0:W0A])
            nc.sync.dma_start(out=x0_sb[0], in_=xt[0, 0])
            for c in range(1, DC):
                nc.gpsimd.dma_start(out=wg_sb[c], in_=wg[c])
                nc.sync.dma_start(out=x0_sb[c], in_=xt[0, c])
            nc.gpsimd.dma_start(out=wg0b, in_=wg[0][:, W0A:])
            for c in range(DC // 2):
                nc.gpsimd.dma_start(out=wu_sb[c], in_=wu[c])
            for c in range(DC // 2, DC):
                nc.sync.dma_start(out=wu_sb[c], in_=wu[c])
            for h in range(HC):
                nc.sync.dma_start(out=wd_sb[:, h, :], in_=wd[h])

            for g in range(NG):
                if g == 0:
                    x_sb = x0_sb
                else:
                    x_sb = [
                        xpool.tile([P, TN], bf16, tag=f"x{c}", name=f"x{g}_{c}")
                        for c in range(DC)
                    ]
                    for c in range(DC):
                        nc.gpsimd.dma_start(out=x_sb[c], in_=xt[g, c])

                ht_sb = hpool.tile([P, HC, TN], bf16, tag="ht")

                if g == 0:
                    # Warm-start: group 0's weights/x stream in while the PE
                    # runs, so a chunk-inner chain (16 matmuls per hh on one
                    # PSUM bank) would leave the PE idle whenever it outruns
                    # DMA arrival. Instead run 6 accumulation chains
                    # (hh 0..5) chunk-OUTER across 6 PSUM banks (borrowing
                    # the pu/po-tagged banks, idle until later): every
                    # arriving chunk feeds 6 matmuls, first for w_gate, then
                    # the same shape again for w_up, with silu(g) staged to
                    # SBUF between the passes. hh 6..10 then take the
                    # standard path below. 6 chains, not 8: 8 interleaved
                    # accumulation groups make the compiler drop FWL
                    # globally (see module docstring).
                    NW = 6
                    sil8 = spool.tile([P, NW, TN], bf16, tag="sil8")
                    for wsb, drain in ((wg_sb, "g"), (wu_sb, "u")):
                        ch_a = psum_pool.tile([P, TN], fp32, tag="pg", name=f"w0{drain}a")
                        ch_b = psum_pool.tile([P, TN], fp32, tag="pg", name=f"w0{drain}b")
                        ch_c = psum_pool.tile([P, TN], fp32, tag="pu", name=f"w0{drain}c")
                        ch_d = psum_pool.tile([P, TN], fp32, tag="pu", name=f"w0{drain}d")
                        ch_ef = psum_pool.tile([P, 2, TN], fp32, tag="po", name=f"w0{drain}ef")
                        chains = [ch_a, ch_b, ch_c, ch_d, ch_ef[:, 0, :], ch_ef[:, 1, :]]
                        for c in range(DC):
                            for hh in range(NW):
                                nc.tensor.matmul(
                                    chains[hh],
                                    wg_slice(c, hh) if drain == "g"
                                    else wsb[c][:, hh * P:(hh + 1) * P],
                                    x_sb[c],
                                    start=(c == 0),
                                    stop=(c == DC - 1),
                                )
                        if drain == "g":
                            for hh in range(NW):
                                sg = spool.tile([P, TN], fp32, tag="sig", name=f"w0sg{hh}")
                                nc.scalar.activation(sg, chains[hh], AF.Sigmoid)
                                nc.vector.tensor_mul(sil8[:, hh, :], chains[hh], sg)
                        else:
                            for hh in range(NW):
                                nc.vector.tensor_mul(
                                    ht_sb[:, hh, :], sil8[:, hh, :], chains[hh]
                                )
                    hh_range = range(NW, HC)
                else:
                    hh_range = range(HC)

                for hh in hh_range:
                    pg = psum_pool.tile([P, TN], fp32, tag="pg")
                    pu = psum_pool.tile([P, TN], fp32, tag="pu")
                    for c in range(DC):
                        nc.tensor.matmul(
                            pg,
                            wg_slice(c, hh),
                            x_sb[c],
                            start=(c == 0),
                            stop=(c == DC - 1),
                        )
                    for c in range(DC):
                        nc.tensor.matmul(
                            pu,
                            wu_sb[c][:, hh * P:(hh + 1) * P],
                            x_sb[c],
                            start=(c == 0),
                            stop=(c == DC - 1),
                        )
                    # silu(g)*u = (g * sigmoid(g)) * u. Each DVE op reads at
                    # most one PSUM operand (HW limit NCC_IBVF027); Silu LUT
                    # isn't in CoreSim so sigmoid+mul keeps this sim-testable.
                    sig = spool.tile([P, TN], fp32, tag="sig")
                    sil = spool.tile([P, TN], fp32, tag="sil")
                    nc.scalar.activation(sig, pg, AF.Sigmoid)
                    nc.vector.tensor_mul(sil, pg, sig)
                    nc.vector.tensor_mul(ht_sb[:, hh, :], sil, pu)

                # Down-projection in dim-halves: po spans 2 PSUM banks and is
                # double-buffered (4 banks total; pg/pu take the other 4), so
                # the PSUM->bf16 casts and the out store of one half overlap
                # the matmuls of the next instead of stalling the PE at every
                # tb boundary (LDWEIGHTS is emitted 1:1 per matmul by the
                # framework, so the extra half split costs no weight loads).
                HD = DIM // 2
                for tb in range(TN // P):
                    t0 = g * TN + tb * P
                    for half in range(2):
                        ot = opool.tile([P, HD], bf16, tag="ot", name=f"ot{g}_{tb}_{half}")
                        po = psum_pool.tile([P, 2, TN], fp32, tag="po")
                        last = g == NG - 1 and tb == TN // P - 1 and half == 1
                        if not last:
                            for hh in range(HC):
                                for do in range(2):
                                    nc.tensor.matmul(
                                        po[:, do, :],
                                        ht_sb[:, hh, tb * P:(tb + 1) * P],
                                        wd_sb[:, hh, (half * 2 + do) * TN:(half * 2 + do + 1) * TN],
                                        start=(hh == 0),
                                        stop=(hh == HC - 1),
                                    )
                        if last:
                            # Final half: run the two do-chains sequentially
                            # (instead of hh-interleaved) and drain each as
                            # it stops, so cast+store of do=0 overlap the
                            # do=1 matmul chain and only one 690ns cast plus
                            # a 128KB store trail the last matmul.
                            for do in range(2):
                                for hh in range(HC):
                                    nc.tensor.matmul(
                                        po[:, do, :],
                                        ht_sb[:, hh, tb * P:(tb + 1) * P],
                                        wd_sb[:, hh, (half * 2 + do) * TN:(half * 2 + do + 1) * TN],
                                        start=(hh == 0),
                                        stop=(hh == HC - 1),
                                        skip_group_check=True,
                                    )
                                nc.vector.tensor_copy(
                                    ot[:, do * TN:(do + 1) * TN], po[:, do, :]
                                )
                                nc.sync.dma_start(
                                    out=out[t0:t0 + P,
                                            half * HD + do * TN:half * HD + (do + 1) * TN],
                                    in_=ot[:, do * TN:(do + 1) * TN],
                                )
                        else:
                            for do in range(2):
                                nc.vector.tensor_copy(ot[:, do * TN:(do + 1) * TN], po[:, do, :])
                            nc.sync.dma_start(
                                out=out[t0:t0 + P, half * HD:(half + 1) * HD], in_=ot
                            )

    nc.compile()
    return nc


def _get_program():
    if not _nc_cache:
        _nc_cache.append(_build_program())
    return _nc_cache[0]


def _in_map_for_core(xe, w_gate, w_up, w_down, e):
    import ml_dtypes

    bf = ml_dtypes.bfloat16
    xtc = np.ascontiguousarray(xe[e].T).astype(bf)           # [DIM, T_E]
    xtc = np.ascontiguousarray(
        xtc.reshape(DC, P, NG, TN).transpose(2, 0, 1, 3)     # [NG, DC, P, TN]
    )
    return {
        "xt": xtc,
        "wg": np.ascontiguousarray(w_gate[e].astype(bf).reshape(DC, P, HIDDEN)),
        "wu": np.ascontiguousarray(w_up[e].astype(bf).reshape(DC, P, HIDDEN)),
        "wd": np.ascontiguousarray(w_down[e].astype(bf).reshape(HC, P, DIM)),
    }


def kernel(x, num_tokens_per_expert, w_gate, w_up, w_down, **_ignored):
    from concourse.bass_utils import run_bass_kernel_spmd

    x = np.asarray(x, dtype=np.float32)
    w_gate = np.asarray(w_gate, dtype=np.float32)
    w_up = np.asarray(w_up, dtype=np.float32)
    w_down = np.asarray(w_down, dtype=np.float32)

    nc = _get_program()

    xe = x.reshape(NUM_EXPERTS, T_E, DIM)
    in_maps = [
        _in_map_for_core(xe, w_gate, w_up, w_down, e) for e in range(NUM_EXPERTS)
    ]

    res = run_bass_kernel_spmd(nc, in_maps, core_ids=list(range(NUM_EXPERTS)))
    outs = [np.asarray(r["out"]).astype(np.float32) for r in res.results]
    return np.concatenate(outs, axis=0)


# revision 35
# speedup vs baseline: 1.0271x; 1.0046x over previous
"""Grouped-experts SwiGLU MLP (DeepseekV3 style) for Trainium2, 8 NeuronCores.

Sharding: expert-parallel. Core e owns expert e's weights and its static
4096-token split. No collectives needed — token routing is the host-side
slice, outputs concatenate back in token order.

Per-core kernel (all matmuls in bf16 with fp32 PSUM accumulation):
  gT[h, t] = wg[d, h].T @ xT[d, t]      (accumulate over 16 d-chunks of 128)
  uT[h, t] = wu[d, h].T @ xT[d, t]
  hT[h, t] = silu(gT) * uT              (ACT sigmoid + DVE muls, stored bf16)
  out[t, d] = hT[h, t].T @ wd[h, d]     (accumulate over 11 h-chunks of 128)

The PE is the bottleneck (bf16 matmul roofline ~900us/core; fp8 DoubleRow
fails the 2e-2 accuracy budget — measured 3.8-6.6% rel err on this problem).
So everything else is shaped to keep the PE issue stream stall-free:

- All HBM tensors are bf16, cast host-side (identical numerics to the old
  fp32->bf16 DMA cast path, RNE both ways). Halves startup DMA bytes: the
  PE's first accumulation chain needs wg + x(group 0), 7.7 MB instead of
  15.7 MB, and the full weight set streams in 48us instead of 97us.
- Inputs land in per-contraction-chunk tiles (16 separate wg/wu/x tiles)
  DMA'd in consumption order, wg/x interleaved across two issue engines, so
  the first matmul starts right after the ~7us engine preamble and group 0
  is paced by DMA arrival rather than blocked on the whole tensor.
- Group 0 warm-start: 6 accumulation chains run chunk-OUTER across 6 PSUM
  banks (borrowing the idle pu/po-tagged banks), so every arriving chunk
  feeds 6 matmuls instead of the PE idling behind one 16-matmul chain.
  NOTE: >=8 interleaved chains (or a bare nc.tensor.ldweights) makes the
  compiler drop Fast Weight Load globally: every LDWEIGHTS doubles to
  ~107ns, matmul cadence goes 215->265ns, +170us whole-kernel. Measured.
- Down-projection runs in dim-halves with po double-buffered (2+2 PSUM
  banks) and bf16 out tiles double-buffered, so the PSUM->bf16 casts
  (690ns each on DVE) and output stores pipeline against the matmuls
  instead of stalling every tb boundary (was ~430ns/tb mid-kernel and
  ~3us/tb in the last group).
- Output is bf16 (adds ~0.02% rel err), upcast to fp32 host-side.

Measured on 8xTRN2 (NTFF profile): 941 us/core, PE matmul-issue floor is
908 us (4224 matmuls x 215 ns), MFU ~0.87. Residual overhead: ~7us engine
preamble, ~5us group-0 DMA pacing, ~4us instruction-fetch bubbles (432ns
per 100 PE instructions), ~5us end drain. fp8 DoubleRow would cut the
floor ~1.44x but fails the 2e-2 budget: e4m3 all-fp8 measures 6.6% rel
err, best hybrid 3.8%; e3m4 (not DoubleRow-capable) 2.0-3.4%.
"""

import numpy as np

NUM_EXPERTS = 8
DIM = 2048
HIDDEN = 1408
T_E = 4096  # tokens per expert (static equal splits)

P = 128
TN = 512              # token group width (matmul moving dim)
NG = T_E // TN        # 8 token groups
DC = DIM // P         # 16 contraction chunks for the up/gate matmuls
HC = HIDDEN // P      # 11 contraction chunks for the down matmul
NDO = DIM // TN       # 4 output-dim blocks of 512

_nc_cache = []


def _build_program():
    import concourse.mybir as mybir
    import concourse.tile as tile
    from concourse import bacc

    fp32 = mybir.dt.float32
    bf16 = mybir.dt.bfloat16
    AF = mybir.ActivationFunctionType

    nc = bacc.Bacc("TRN2", target_bir_lowering=False, debug=False)

    xt = nc.dram_tensor("xt", [NG, DC, P, TN], bf16, kind="ExternalInput")
    wg = nc.dram_tensor("wg", [DC, P, HIDDEN], bf16, kind="ExternalInput")
    wu = nc.dram_tensor("wu", [DC, P, HIDDEN], bf16, kind="ExternalInput")
    wd = nc.dram_tensor("wd", [HC, P, DIM], bf16, kind="ExternalInput")
    out = nc.dram_tensor("out", [T_E, DIM], bf16, kind="ExternalOutput")

    with tile.TileContext(nc) as tc:
        with (
            tc.tile_pool(name="wpool", bufs=1) as wpool,
            tc.tile_pool(name="xpool", bufs=2) as xpool,
            tc.tile_pool(name="hpool", bufs=1) as hpool,
            tc.tile_pool(name="spool", bufs=1) as spool,
            tc.tile_pool(name="opool", bufs=2) as opool,
            tc.tile_pool(name="psum", bufs=2, space="PSUM") as psum_pool,
        ):
            # Every wg chunk is split at col 768: the warm-start chains only
            # read hh 0..5 (the A half), so 192KB slices gate the paced
            # phase (3MB critical instead of 5.6MB) and the B halves (for
            # hh 6..10, first needed ~60us in) stream after wu.
            W0A = 6 * P
            wgA = [
                wpool.tile([P, W0A], bf16, tag=f"wgA{c}", name=f"wgA{c}")
                for c in range(DC)
            ]
            wgB = [
                wpool.tile([P, HIDDEN - W0A], bf16, tag=f"wgB{c}", name=f"wgB{c}")
                for c in range(DC)
            ]

            def wg_slice(c, hh):
                if hh < 6:
                    return wgA[c][:, hh * P:(hh + 1) * P]
                return wgB[c][:, (hh - 6) * P:(hh - 5) * P]
            wu_sb = [
                wpool.tile([P, HIDDEN], bf16, tag=f"wu{c}", name=f"wu{c}")
                for c in range(DC)
            ]
            wd_sb = wpool.tile([P, HC, DIM], bf16, tag="wd")
            x0_sb = [
                xpool.tile([P, TN], bf16, tag=f"x{c}", name=f"x0_{c}")
                for c in range(DC)
            ]
            # Consumption-ordered loads, interleaved across two issue engines
            # so descriptor emission (~0.65us/DMA/engine) doesn't pace the
            # queues. wg/x0 pace the warm-start PE chains, so they go first;
            # wu (needed ~32us in, by the u-pass) is split across both
            # engines behind them, then wd (needed ~75us in).
            for c in range(DC):
                nc.gpsimd.dma_start(out=wgA[c], in_=wg[c][:, 0:W0A])
                nc.sync.dma_start(out=x0_sb[c], in_=xt[0, c])
            for c in range(DC // 2):
                nc.gpsimd.dma_start(out=wu_sb[c], in_=wu[c])
            for c in range(DC // 2, DC):
                nc.sync.dma_start(out=wu_sb[c], in_=wu[c])
            for c in range(DC):
                nc.gpsimd.dma_start(out=wgB[c], in_=wg[c][:, W0A:])
            for h in range(HC):
                nc.sync.dma_start(out=wd_sb[:, h, :], in_=wd[h])

            for g in range(NG):
                if g == 0:
                    x_sb = x0_sb
                else:
                    x_sb = [
                        xpool.tile([P, TN], bf16, tag=f"x{c}", name=f"x{g}_{c}")
                        for c in range(DC)
                    ]
                    for c in range(DC):
                        nc.gpsimd.dma_start(out=x_sb[c], in_=xt[g, c])

                ht_sb = hpool.tile([P, HC, TN], bf16, tag="ht")

                if g == 0:
                    # Warm-start: group 0's weights/x stream in while the PE
                    # runs, so a chunk-inner chain (16 matmuls per hh on one
                    # PSUM bank) would leave the PE idle whenever it outruns
                    # DMA arrival. Instead run 6 accumulation chains
                    # (hh 0..5) chunk-OUTER across 6 PSUM banks (borrowing
                    # the pu/po-tagged banks, idle until later): every
                    # arriving chunk feeds 6 matmuls, first for w_gate, then
                    # the same shape again for w_up, with silu(g) staged to
                    # SBUF between the passes. hh 6..10 then take the
                    # standard path below. 6 chains, not 8: 8 interleaved
                    # accumulation groups make the compiler drop FWL
                    # globally (see module docstring).
                    NW = 6
                    sil8 = spool.tile([P, NW, TN], bf16, tag="sil8")
                    for wsb, drain in ((None, "g"), (wu_sb, "u")):
                        ch_a = psum_pool.tile([P, TN], fp32, tag="pg", name=f"w0{drain}a")
                        ch_b = psum_pool.tile([P, TN], fp32, tag="pg", name=f"w0{drain}b")
                        ch_c = psum_pool.tile([P, TN], fp32, tag="pu", name=f"w0{drain}c")
                        ch_d = psum_pool.tile([P, TN], fp32, tag="pu", name=f"w0{drain}d")
                        ch_ef = psum_pool.tile([P, 2, TN], fp32, tag="po", name=f"w0{drain}ef")
                        chains = [ch_a, ch_b, ch_c, ch_d, ch_ef[:, 0, :], ch_ef[:, 1, :]]
                        for c in range(DC):
                            for hh in range(NW):
                                nc.tensor.matmul(
                                    chains[hh],
                                    wg_slice(c, hh) if drain == "g"
                                    else wsb[c][:, hh * P:(hh + 1) * P],
                                    x_sb[c],
                                    start=(c == 0),
                                    stop=(c == DC - 1),
                                )
                        if drain == "g":
                            for hh in range(NW):
                                sg = spool.tile([P, TN], fp32, tag="sig", name=f"w0sg{hh}")
                                nc.scalar.activation(sg, chains[hh], AF.Sigmoid)
                                nc.vector.tensor_mul(sil8[:, hh, :], chains[hh], sg)
                        else:
                            for hh in range(NW):
                                nc.vector.tensor_mul(
                                    ht_sb[:, hh, :], sil8[:, hh, :], chains[hh]
                                )
                    hh_range = range(NW, HC)
                else:
                    hh_range = range(HC)

                for hh in hh_range:
                    pg = psum_pool.tile([P, TN], fp32, tag="pg")
                    pu = psum_pool.tile([P, TN], fp32, tag="pu")
                    for c in range(DC):
                        nc.tensor.matmul(
                            pg,
                            wg_slice(c, hh),
                            x_sb[c],
                            start=(c == 0),
                            stop=(c == DC - 1),
                        )
                    for c in range(DC):
                        nc.tensor.matmul(
                            pu,
                            wu_sb[c][:, hh * P:(hh + 1) * P],
                            x_sb[c],
                            start=(c == 0),
                            stop=(c == DC - 1),
                        )
                    # silu(g)*u = (g * sigmoid(g)) * u. Each DVE op reads at
                    # most one PSUM operand (HW limit NCC_IBVF027); Silu LUT
                    # isn't in CoreSim so sigmoid+mul keeps this sim-testable.
                    sig = spool.tile([P, TN], fp32, tag="sig")
                    sil = spool.tile([P, TN], fp32, tag="sil")
                    nc.scalar.activation(sig, pg, AF.Sigmoid)
                    nc.vector.tensor_mul(sil, pg, sig)
                    nc.vector.tensor_mul(ht_sb[:, hh, :], sil, pu)

                # Down-projection in dim-halves: po spans 2 PSUM banks and is
                # double-buffered (4 banks total; pg/pu take the other 4), so
                # the PSUM->bf16 casts and the out store of one half overlap
                # the matmuls of the next instead of stalling the PE at every
                # tb boundary (LDWEIGHTS is emitted 1:1 per matmul by the
                # framework, so the extra half split costs no weight loads).
                HD = DIM // 2
                for tb in range(TN // P):
                    t0 = g * TN + tb * P
                    for half in range(2):
                        ot = opool.tile([P, HD], bf16, tag="ot", name=f"ot{g}_{tb}_{half}")
                        po = psum_pool.tile([P, 2, TN], fp32, tag="po")
                        last = g == NG - 1 and tb == TN // P - 1 and half == 1
                        if not last:
                            for hh in range(HC):
                                for do in range(2):
                                    nc.tensor.matmul(
                                        po[:, do, :],
                                        ht_sb[:, hh, tb * P:(tb + 1) * P],
                                        wd_sb[:, hh, (half * 2 + do) * TN:(half * 2 + do + 1) * TN],
                                        start=(hh == 0),
                                        stop=(hh == HC - 1),
                                    )
                        if last:
                            # Final half: run the two do-chains sequentially
                            # (instead of hh-interleaved) and drain each as
                            # it stops, so cast+store of do=0 overlap the
                            # do=1 matmul chain and only one 690ns cast plus
                            # a 128KB store trail the last matmul.
                            for do in range(2):
                                for hh in range(HC):
                                    nc.tensor.matmul(
                                        po[:, do, :],
                                        ht_sb[:, hh, tb * P:(tb + 1) * P],
                                        wd_sb[:, hh, (half * 2 + do) * TN:(half * 2 + do + 1) * TN],
                                        start=(hh == 0),
                                        stop=(hh == HC - 1),
                                        skip_group_check=True,
                                    )
                                nc.vector.tensor_copy(
                                    ot[:, do * TN:(do + 1) * TN], po[:, do, :]
                                )
                                nc.sync.dma_start(
                                    out=out[t0:t0 + P,
                                            half * HD + do * TN:half * HD + (do + 1) * TN],
                                    in_=ot[:, do * TN:(do + 1) * TN],
                                )
                        else:
                            for do in range(2):
                                nc.vector.tensor_copy(ot[:, do * TN:(do + 1) * TN], po[:, do, :])
                            nc.sync.dma_start(
                                out=out[t0:t0 + P, half * HD:(half + 1) * HD], in_=ot
                            )

    nc.compile()
    return nc


def _get_program():
    if not _nc_cache:
        _nc_cache.append(_build_program())
    return _nc_cache[0]


def _in_map_for_core(xe, w_gate, w_up, w_down, e):
    import ml_dtypes

    bf = ml_dtypes.bfloat16
    xtc = np.ascontiguousarray(xe[e].T).astype(bf)           # [DIM, T_E]
    xtc = np.ascontiguousarray(
        xtc.reshape(DC, P, NG, TN).transpose(2, 0, 1, 3)     # [NG, DC, P, TN]
    )
    return {
        "xt": xtc,
        "wg": np.ascontiguousarray(w_gate[e].astype(bf).reshape(DC, P, HIDDEN)),
        "wu": np.ascontiguousarray(w_up[e].astype(bf).reshape(DC, P, HIDDEN)),
        "wd": np.ascontiguousarray(w_down[e].astype(bf).reshape(HC, P, DIM)),
    }


def kernel(x, num_tokens_per_expert, w_gate, w_up, w_down, **_ignored):
    from concourse.bass_utils import run_bass_kernel_spmd

    x = np.asarray(x, dtype=np.float32)
    w_gate = np.asarray(w_gate, dtype=np.float32)
    w_up = np.asarray(w_up, dtype=np.float32)
    w_down = np.asarray(w_down, dtype=np.float32)

    nc = _get_program()

    xe = x.reshape(NUM_EXPERTS, T_E, DIM)
    in_maps = [
        _in_map_for_core(xe, w_gate, w_up, w_down, e) for e in range(NUM_EXPERTS)
    ]

    res = run_bass_kernel_spmd(nc, in_maps, core_ids=list(range(NUM_EXPERTS)))
    outs = [np.asarray(r["out"]).astype(np.float32) for r in res.results]
    return np.concatenate(outs, axis=0)
